# revision 25
# baseline (speedup 1.0000x reference)
"""Deterministic MoE router kernel for Trainium2 (8 NeuronCores, SPMD).

Computes, for hidden_states [4, 4096, 2048] f32 and gate_w [128, 2048] f32:
  router_logits  = hidden @ gate_w.T            [4, 4096, 128] f32
  expert_indices = top-6 (deterministic ties)   [4, 4096, 6]   int32
  expert_weights = softmax(top-6 orig logits)   [4, 4096, 6]   f32

Sharding: data-parallel over tokens (B*S = 16384 -> 2048 tokens/core); the
tiny gate weight is replicated.

Matmul strategy: fp16 hi/lo decomposition. hidden = hi + lo and gate =
ghi + glo with hi/lo in fp16 (11-bit mantissas, products exact in fp32
PSUM). logits ~= ghi'hi + glo'hi + ghi'lo -- three single-pass fp16
matmuls at 1 cyc/row instead of one fp32 matmul at 4 cyc/row. Max abs
error ~3e-6 (vs ~1e-6 for fp32). The same 16 MB of hidden bytes move from
HBM (2 x fp16 vs 1 x fp32), so DMA cost is unchanged and the kernel sits
on the ridge: PE ~47us ~= DMA ~47us per core.

Top-k exactness: the device reports the top-8 adjusted values per token.
Any token whose adjacent top-7 gaps dip below 2e-5 (~40 sigma above the
fp16-split error; ~10 tokens out of 16k) is re-resolved exactly on the
host with one fp32 row matmul. Everything else is provably stable.

Per-core device layout:
  - hidden shard staged [group][128p][chunk][hi|lo][512tok] fp16 so every
    DMA is a contiguous 8KB-per-partition block
  - PSUM accumulates logitsT [128e, 512tok] over 48 fp16 matmuls/group
  - PSUM->SBUF copy folds in the tie-breaker subtract (per-partition)
  - PE transposes back to [tok, e]; ACT stages the logits copy; DVE
    max/max_index produce the top-8 values + indices
  - outputs: adjusted logits (host adds the tie row back) and a packed
    [8 idx u32 | 8 max f32] per-tile stage; the 6-value softmax is a
    trivial host epilogue mirroring the reference formula
"""

import sys

for _p in ("/opt/trn_rl_repo",):
    if _p not in sys.path:
        sys.path.insert(0, _p)

import numpy as np

import concourse.bacc as bacc
import concourse.mybir as mybir
import concourse.tile as tile
from concourse.bass_utils import run_bass_kernel_spmd

F32 = mybir.dt.float32
F16 = mybir.dt.float16
U32 = mybir.dt.uint32

B, S, H, E, K = 4, 4096, 2048, 128, 6
N_CORES = 8
N_TOK = B * S
T = N_TOK // N_CORES            # tokens per core (2048)
NCH = H // 128                  # contraction chunks (16)
GRP = 512                       # tokens per PSUM accumulation group
NG = T // GRP                   # groups per core (4)
NT_T = T // 128                 # token tiles per core (16)
PK = 16                         # stage stride: 8 idx u32 | 8 max f32
GAP_TAU = 2e-5                  # host re-resolve threshold on adjacent gaps

TIE = np.arange(E, dtype=np.float32) * np.float32(1e-9)

_cache = {}


def _build():
    nc = bacc.Bacc("TRN2", target_bir_lowering=False, debug=False)

    # [group][128p][chunk][hi|lo][512tok] fp16
    hid = nc.dram_tensor("hid", [NG, 128, NCH, 2, GRP], F16, kind="ExternalInput")
    gwhi_d = nc.dram_tensor("gwhi", [H, E], F16, kind="ExternalInput")
    gwlo_d = nc.dram_tensor("gwlo", [H, E], F16, kind="ExternalInput")
    iden_d = nc.dram_tensor("iden", [128, 128], F32, kind="ExternalInput")
    ntie_d = nc.dram_tensor("ntie", [128, 1], F32, kind="ExternalInput")

    adj_d = nc.dram_tensor("adj", [T, E], F32, kind="ExternalOutput")
    iw_d = nc.dram_tensor("iw", [128, NT_T * PK], U32, kind="ExternalOutput")

    add = mybir.AluOpType.add

    NQ = 4                      # DMA quarters per group (4 chunks each)
    CPQ = NCH // NQ

    with tile.TileContext(nc) as tc:
        with (
            tc.tile_pool(name="const", bufs=1) as cpool,
            tc.tile_pool(name="htg", bufs=NG) as htpool,
            tc.tile_pool(name="lgT", bufs=3) as lgtpool,
            tc.tile_pool(name="stage", bufs=1) as spool,
            tc.tile_pool(name="work", bufs=4) as wpool,
            tc.tile_pool(name="psmm", bufs=3, space="PSUM") as psmm,
            tc.tile_pool(name="pstr", bufs=4, space="PSUM") as pstr,
            tc.tile_pool(name="psdummy", bufs=1, space="PSUM") as psdummy,
        ):
            # hidden group 0 quarters interleaved with the gate quarters they
            # unlock, so the first matmul's data is in flight immediately
            gwhi = cpool.tile([128, NCH * E], F16)
            gwlo = cpool.tile([128, NCH * E], F16)
            ghi_r = gwhi_d.ap().rearrange("(c p) e -> p c e", p=128)
            glo_r = gwlo_d.ap().rearrange("(c p) e -> p c e", p=128)
            iden = cpool.tile([128, 128], F32)
            ntie = cpool.tile([128, 1], F32)

            hts = {}
            for g in range(NG):
                t = htpool.tile([128, NCH * 2 * GRP], F16, tag="htg", name=f"ht_{g}")
                hts[g] = t.rearrange("p (c l w) -> p c l w", c=NCH, l=2)

            # all input issues on SP (cross-engine DMA issue shares Tile's
            # 8-semaphore rotation and couples the engines — keep one stream).
            # group-0 quarters interleave with the gate quarters they unlock.
            for q in range(NQ):
                nc.sync.dma_start(
                    hts[0][:, q * CPQ : (q + 1) * CPQ],
                    hid[0, :, q * CPQ : (q + 1) * CPQ],
                )
                cs = slice(q * CPQ * E, (q + 1) * CPQ * E)
                nc.sync.dma_start(gwhi[:, cs], ghi_r[:, q * CPQ : (q + 1) * CPQ])
                nc.sync.dma_start(gwlo[:, cs], glo_r[:, q * CPQ : (q + 1) * CPQ])
            nc.sync.dma_start(iden[:], iden_d[:])
            nc.sync.dma_start(ntie[:], ntie_d[:])
            for g in range(1, NG):
                for q in range(NQ):
                    nc.sync.dma_start(
                        hts[g][:, q * CPQ : (q + 1) * CPQ],
                        hid[g, :, q * CPQ : (q + 1) * CPQ],
                    )

            # warm up the PE clock gate while the first DMAs land
            dummy = cpool.tile([128, GRP], F16)
            nc.vector.memset(dummy[:], 0.0)
            psd = psdummy.tile([128, GRP], F32)
            for _ in range(6):
                nc.tensor.matmul(
                    psd[:], dummy[:, 0:E], dummy[:], start=True, stop=True,
                    skip_group_check=True,
                )

            lg_stage = spool.tile([128, NT_T * E], F32)
            iw_stage = spool.tile([128, NT_T * PK], U32)

            def emit_post(g):
                # per-tile tie-subtract PSUM copy -> transpose -> stage + top-8
                lgT = lgtpool.tile([128, GRP], F32, tag="lgT", name=f"lgT_{g}")
                for s in range(GRP // 128):
                    ti = g * (GRP // 128) + s  # token tile index
                    o = ti * PK
                    sl = slice(s * 128, (s + 1) * 128)
                    nc.vector.tensor_scalar(
                        lgT[:, sl], pss[g][:, sl], ntie[:, 0:1], None, op0=add
                    )
                    pt = pstr.tile([128, 128], F32, tag="tr", name=f"pt_{ti}")
                    nc.tensor.transpose(pt[:], lgT[:, sl], iden[:])
                    # adjusted logits [tok, e] -> stage (host adds tie back)
                    nc.scalar.copy(lg_stage[:, ti * E : (ti + 1) * E], pt[:])
                    mxs = iw_stage[:, o + 8 : o + 16].bitcast(F32)
                    nc.vector.max(mxs, pt[:])
                    nc.vector.max_index(iw_stage[:, o : o + 8], mxs, pt[:])
                # flush adjusted logits once staged (last group in halves so
                # the final transfer waits on fewer copies)
                adj_g = adj_d.ap().rearrange("(g t p) e -> g p t e", p=128, g=NG)[g]
                if g == NG - 1:
                    nc.sync.dma_start(
                        adj_g[:, 0:2], lg_stage[:, g * 4 * E : (g * 4 + 2) * E]
                    )
                    nc.sync.dma_start(
                        adj_g[:, 2:4], lg_stage[:, (g * 4 + 2) * E : (g + 1) * 4 * E]
                    )
                else:
                    nc.sync.dma_start(
                        adj_g, lg_stage[:, g * 4 * E : (g + 1) * 4 * E]
                    )

            pss = {}
            for g in range(NG):
                ps_acc = psmm.tile([128, GRP], F32, tag="mm", name=f"ps_{g}")
                pss[g] = ps_acc
                hv = hts[g]  # [p, c, hi|lo, tok] view
                for c in range(NCH):
                    se = slice(c * E, (c + 1) * E)
                    nc.tensor.matmul(
                        ps_acc[:], gwhi[:, se], hv[:, c, 0],
                        start=(c == 0), stop=False,
                    )
                    nc.tensor.matmul(
                        ps_acc[:], gwhi[:, se], hv[:, c, 1],
                        start=False, stop=False,
                    )
                    nc.tensor.matmul(
                        ps_acc[:], gwlo[:, se], hv[:, c, 0],
                        start=False, stop=(c == NCH - 1),
                    )
                # defer group g-1's post work behind group g's matmuls so the
                # PE queue head is never blocked on DVE at a group boundary
                if g > 0:
                    emit_post(g - 1)
            emit_post(NG - 1)

            # two half-flushes: the final transfer only waits on tiles 8-15
            half = NT_T * PK // 2
            nc.sync.dma_start(iw_d.ap()[:, 0:half], iw_stage[:, 0:half])
            nc.sync.dma_start(iw_d.ap()[:, half:], iw_stage[:, half:])

    nc.compile()
    return nc


def _get_nc():
    if "nc" not in _cache:
        _cache["nc"] = _build()
    return _cache["nc"]


def _split16(a):
    hi = a.astype(np.float16)
    lo = (a - hi.astype(np.float32)).astype(np.float16)
    return hi, lo


def _host_inputs(flat, gate_w):
    gwt = np.ascontiguousarray(gate_w.T)                 # [H, E] f32
    gwhi, gwlo = _split16(gwt)
    iden = np.eye(128, dtype=np.float32)
    ntie = (-TIE[:128]).reshape(128, 1).copy()
    in_maps = []
    for i in range(N_CORES):
        shard_t = flat[i * T : (i + 1) * T, :].T          # [H, T] (view)
        hi, lo = _split16(np.ascontiguousarray(shard_t))
        # [H, T] -> [NCH, 128, NG, GRP] -> [NG, 128, NCH, GRP]
        hi4 = hi.reshape(NCH, 128, NG, GRP).transpose(2, 1, 0, 3)
        lo4 = lo.reshape(NCH, 128, NG, GRP).transpose(2, 1, 0, 3)
        hid = np.ascontiguousarray(
            np.stack([hi4, lo4], axis=3)                  # [NG,128,NCH,2,GRP]
        )
        in_maps.append(
            {"hid": hid, "gwhi": gwhi, "gwlo": gwlo, "iden": iden, "ntie": ntie}
        )
    return in_maps


def _gather(results, flat, gate_w):
    adj = np.concatenate([r["adj"] for r in results], axis=0)     # [N_TOK, E]
    idx_parts, mx_parts = [], []
    for r in results:
        iw = r["iw"].reshape(128, NT_T, PK)
        idx_parts.append(iw[:, :, 0:8].transpose(1, 0, 2).reshape(T, 8))
        mx_parts.append(
            iw[:, :, 8:16].view(np.float32).transpose(1, 0, 2).reshape(T, 8)
        )
    idx8 = np.concatenate(idx_parts, axis=0).astype(np.int64)
    mx = np.concatenate(mx_parts, axis=0)                          # [N_TOK, 8]
    idx = idx8[:, :K].copy()

    # host epilogue: original top-6 values and their softmax (ref formula)
    vals = mx[:, :K] + TIE[idx]
    # host re-resolution of tokens with tight adjacent gaps in the top-7
    gaps = mx[:, :7] - mx[:, 1:8]
    risky = np.where(gaps.min(axis=1) < GAP_TAU)[0]
    if len(risky):
        rows = flat[risky] @ gate_w.T                              # [R, E] f32
        adjr = rows - TIE[None, :]
        order = np.argsort(-adjr, axis=1, kind="stable")[:, :K]
        adj[risky] = adjr
        idx[risky] = order
        vals[risky] = np.take_along_axis(rows, order, axis=1)

    e = np.exp(vals - vals.max(axis=1, keepdims=True))
    w = (e / e.sum(axis=1, keepdims=True)).astype(np.float32)
    logits = adj + TIE[None, :]
    return (
        logits.reshape(B, S, E).astype(np.float32),
        idx.reshape(B, S, K).astype(np.int32),
        w.reshape(B, S, K).astype(np.float32),
    )


def run(hidden_states, gate_w, trace=False, **trace_kwargs):
    nc = _get_nc()
    flat = np.ascontiguousarray(hidden_states, dtype=np.float32).reshape(N_TOK, H)
    gw = np.ascontiguousarray(gate_w, dtype=np.float32).reshape(E, H)
    in_maps = _host_inputs(flat, gw)
    res = run_bass_kernel_spmd(
        nc, in_maps, list(range(N_CORES)), trace=trace, **trace_kwargs
    )
    return _gather(res.results, flat, gw), res


def kernel(hidden_states, gate_w):
    out, _ = run(hidden_states, gate_w)
    return out


# revision 27
# speedup vs baseline: 1.0973x; 1.0973x over previous
"""Deterministic MoE router kernel for Trainium2 (8 NeuronCores, SPMD).

Computes, for hidden_states [4, 4096, 2048] f32 and gate_w [128, 2048] f32:
  router_logits  = hidden @ gate_w.T            [4, 4096, 128] f32
  expert_indices = top-6 (deterministic ties)   [4, 4096, 6]   int32
  expert_weights = softmax(top-6 orig logits)   [4, 4096, 6]   f32

Sharding: data-parallel over tokens (B*S = 16384 -> 2048 tokens/core); the
tiny gate weight is replicated.

Matmul strategy: fp16 hi/lo decomposition. hidden = hi + lo and gate =
ghi + glo with hi/lo in fp16 (11-bit mantissas, products exact in fp32
PSUM). logits ~= ghi'hi + glo'hi + ghi'lo -- three single-pass fp16
matmuls at 1 cyc/row instead of one fp32 matmul at 4 cyc/row. Max abs
error ~3e-6 (vs ~1e-6 for fp32). The same 16 MB of hidden bytes move from
HBM (2 x fp16 vs 1 x fp32), so DMA cost is unchanged and the kernel sits
on the ridge: PE ~47us ~= DMA ~47us per core.

Top-k exactness: the device reports the top-8 adjusted values per token.
Any token whose adjacent top-7 gaps dip below 2e-5 (~40 sigma above the
fp16-split error; ~10 tokens out of 16k) is re-resolved exactly on the
host with one fp32 row matmul. Everything else is provably stable.

Per-core device layout:
  - hidden shard staged [group][128p][chunk][hi|lo][512tok] fp16 so every
    DMA is a contiguous 8KB-per-partition block
  - PSUM accumulates logitsT [128e, 512tok] over 48 fp16 matmuls/group
  - PSUM->SBUF copy folds in the tie-breaker subtract (per-partition)
  - PE transposes back to [tok, e]; ACT stages the logits copy; DVE
    max/max_index produce the top-8 values + indices
  - outputs: adjusted logits (host adds the tie row back) and a packed
    [8 idx u32 | 8 max f32] per-tile stage; the 6-value softmax is a
    trivial host epilogue mirroring the reference formula
"""

import sys

for _p in ("/opt/trn_rl_repo",):
    if _p not in sys.path:
        sys.path.insert(0, _p)

import numpy as np

import concourse.bacc as bacc
import concourse.mybir as mybir
import concourse.tile as tile
from concourse.bass_utils import run_bass_kernel_spmd

F32 = mybir.dt.float32
F16 = mybir.dt.float16
U32 = mybir.dt.uint32

B, S, H, E, K = 4, 4096, 2048, 128, 6
N_CORES = 8
N_TOK = B * S
T = N_TOK // N_CORES            # tokens per core (2048)
NCH = H // 128                  # contraction chunks (16)
GRP = 512                       # tokens per PSUM accumulation group
NG = T // GRP                   # groups per core (4)
NT_T = T // 128                 # token tiles per core (16)
PK = 16                         # stage stride: 8 idx u32 | 8 max f32
GAP_TAU = 2e-5                  # host re-resolve threshold on adjacent gaps

TIE = np.arange(E, dtype=np.float32) * np.float32(1e-9)

_cache = {}


def _build():
    nc = bacc.Bacc("TRN2", target_bir_lowering=False, debug=False)

    # [group][128p][chunk][hi|lo][512tok] fp16
    hid = nc.dram_tensor("hid", [NG, 128, NCH, 2, GRP], F16, kind="ExternalInput")
    gwhi_d = nc.dram_tensor("gwhi", [H, E], F16, kind="ExternalInput")
    gwlo_d = nc.dram_tensor("gwlo", [H, E], F16, kind="ExternalInput")
    iden_d = nc.dram_tensor("iden", [128, 128], F32, kind="ExternalInput")
    ntie_d = nc.dram_tensor("ntie", [128, 1], F32, kind="ExternalInput")

    adj_d = nc.dram_tensor("adj", [T, E], F32, kind="ExternalOutput")
    iw_d = nc.dram_tensor("iw", [128, NT_T * PK], U32, kind="ExternalOutput")

    add = mybir.AluOpType.add

    NQ = 4                      # DMA quarters per group (4 chunks each)
    CPQ = NCH // NQ

    with tile.TileContext(nc) as tc:
        with (
            tc.tile_pool(name="const", bufs=1) as cpool,
            tc.tile_pool(name="htg", bufs=NG) as htpool,
            tc.tile_pool(name="lgT", bufs=3) as lgtpool,
            tc.tile_pool(name="stage", bufs=1) as spool,
            tc.tile_pool(name="work", bufs=4) as wpool,
            tc.tile_pool(name="psmm", bufs=3, space="PSUM") as psmm,
            tc.tile_pool(name="pstr", bufs=4, space="PSUM") as pstr,
            tc.tile_pool(name="psdummy", bufs=1, space="PSUM") as psdummy,
        ):
            # hidden group 0 quarters interleaved with the gate quarters they
            # unlock, so the first matmul's data is in flight immediately
            gwhi = cpool.tile([128, NCH * E], F16)
            gwlo = cpool.tile([128, NCH * E], F16)
            ghi_r = gwhi_d.ap().rearrange("(c p) e -> p c e", p=128)
            glo_r = gwlo_d.ap().rearrange("(c p) e -> p c e", p=128)
            iden = cpool.tile([128, 128], F32)
            ntie = cpool.tile([128, 1], F32)

            hts = {}
            for g in range(NG):
                t = htpool.tile([128, NCH * 2 * GRP], F16, tag="htg", name=f"ht_{g}")
                hts[g] = t.rearrange("p (c l w) -> p c l w", c=NCH, l=2)

            # all input issues on SP (cross-engine DMA issue shares Tile's
            # 8-semaphore rotation and couples the engines — keep one stream).
            # group-0 quarters interleave with the gate quarters they unlock.
            for q in range(NQ):
                nc.sync.dma_start(
                    hts[0][:, q * CPQ : (q + 1) * CPQ],
                    hid[0, :, q * CPQ : (q + 1) * CPQ],
                )
                cs = slice(q * CPQ * E, (q + 1) * CPQ * E)
                nc.sync.dma_start(gwhi[:, cs], ghi_r[:, q * CPQ : (q + 1) * CPQ])
                nc.sync.dma_start(gwlo[:, cs], glo_r[:, q * CPQ : (q + 1) * CPQ])
            nc.sync.dma_start(iden[:], iden_d[:])
            nc.sync.dma_start(ntie[:], ntie_d[:])
            for g in range(1, NG):
                for q in range(NQ):
                    nc.sync.dma_start(
                        hts[g][:, q * CPQ : (q + 1) * CPQ],
                        hid[g, :, q * CPQ : (q + 1) * CPQ],
                    )

            # warm up the PE clock gate while the first DMAs land
            dummy = cpool.tile([128, GRP], F16)
            nc.vector.memset(dummy[:], 0.0)
            psd = psdummy.tile([128, GRP], F32)
            for _ in range(8):
                nc.tensor.matmul(
                    psd[:], dummy[:, 0:E], dummy[:], start=True, stop=True,
                    skip_group_check=True,
                )

            lg_stage = spool.tile([128, NT_T * E], F32)
            iw_stage = spool.tile([128, NT_T * PK], U32)

            def emit_post(g):
                # tie-subtract PSUM copy, transpose back, stage + top-8
                lgT = lgtpool.tile([128, GRP], F32, tag="lgT", name=f"lgT_{g}")
                nc.vector.tensor_scalar(
                    lgT[:], pss[g][:], ntie[:, 0:1], None, op0=add
                )
                for s in range(GRP // 128):
                    ti = g * (GRP // 128) + s  # token tile index
                    o = ti * PK
                    pt = pstr.tile([128, 128], F32, tag="tr", name=f"pt_{ti}")
                    nc.tensor.transpose(
                        pt[:], lgT[:, s * 128 : (s + 1) * 128], iden[:]
                    )
                    # adjusted logits [tok, e] -> stage (host adds tie back)
                    nc.scalar.copy(lg_stage[:, ti * E : (ti + 1) * E], pt[:])
                    mxs = iw_stage[:, o + 8 : o + 16].bitcast(F32)
                    nc.vector.max(mxs, pt[:])
                    nc.vector.max_index(iw_stage[:, o : o + 8], mxs, pt[:])
                # flush adjusted logits for this group once staged
                nc.sync.dma_start(
                    adj_d.ap().rearrange("(g t p) e -> g p t e", p=128, g=NG)[g],
                    lg_stage[:, g * 4 * E : (g + 1) * 4 * E],
                )

            pss = {}
            for g in range(NG):
                ps_acc = psmm.tile([128, GRP], F32, tag="mm", name=f"ps_{g}")
                pss[g] = ps_acc
                hv = hts[g]  # [p, c, hi|lo, tok] view
                for c in range(NCH):
                    se = slice(c * E, (c + 1) * E)
                    nc.tensor.matmul(
                        ps_acc[:], gwhi[:, se], hv[:, c, 0],
                        start=(c == 0), stop=False,
                    )
                    nc.tensor.matmul(
                        ps_acc[:], gwhi[:, se], hv[:, c, 1],
                        start=False, stop=False,
                    )
                    nc.tensor.matmul(
                        ps_acc[:], gwlo[:, se], hv[:, c, 0],
                        start=False, stop=(c == NCH - 1),
                    )
                # defer group g-1's post work behind group g's matmuls so the
                # PE queue head is never blocked on DVE at a group boundary
                if g > 0:
                    emit_post(g - 1)
            emit_post(NG - 1)

            # two half-flushes: the final transfer only waits on tiles 8-15
            half = NT_T * PK // 2
            nc.sync.dma_start(iw_d.ap()[:, 0:half], iw_stage[:, 0:half])
            nc.sync.dma_start(iw_d.ap()[:, half:], iw_stage[:, half:])

    nc.compile()
    return nc


def _get_nc():
    if "nc" not in _cache:
        _cache["nc"] = _build()
    return _cache["nc"]


def _split16(a):
    hi = a.astype(np.float16)
    lo = (a - hi.astype(np.float32)).astype(np.float16)
    return hi, lo


def _host_inputs(flat, gate_w):
    gwt = np.ascontiguousarray(gate_w.T)                 # [H, E] f32
    gwhi, gwlo = _split16(gwt)
    iden = np.eye(128, dtype=np.float32)
    ntie = (-TIE[:128]).reshape(128, 1).copy()
    in_maps = []
    for i in range(N_CORES):
        shard_t = flat[i * T : (i + 1) * T, :].T          # [H, T] (view)
        hi, lo = _split16(np.ascontiguousarray(shard_t))
        # [H, T] -> [NCH, 128, NG, GRP] -> [NG, 128, NCH, GRP]
        hi4 = hi.reshape(NCH, 128, NG, GRP).transpose(2, 1, 0, 3)
        lo4 = lo.reshape(NCH, 128, NG, GRP).transpose(2, 1, 0, 3)
        hid = np.ascontiguousarray(
            np.stack([hi4, lo4], axis=3)                  # [NG,128,NCH,2,GRP]
        )
        in_maps.append(
            {"hid": hid, "gwhi": gwhi, "gwlo": gwlo, "iden": iden, "ntie": ntie}
        )
    return in_maps


def _gather(results, flat, gate_w):
    adj = np.concatenate([r["adj"] for r in results], axis=0)     # [N_TOK, E]
    idx_parts, mx_parts = [], []
    for r in results:
        iw = r["iw"].reshape(128, NT_T, PK)
        idx_parts.append(iw[:, :, 0:8].transpose(1, 0, 2).reshape(T, 8))
        mx_parts.append(
            iw[:, :, 8:16].view(np.float32).transpose(1, 0, 2).reshape(T, 8)
        )
    idx8 = np.concatenate(idx_parts, axis=0).astype(np.int64)
    mx = np.concatenate(mx_parts, axis=0)                          # [N_TOK, 8]
    idx = idx8[:, :K].copy()

    # host epilogue: original top-6 values and their softmax (ref formula)
    vals = mx[:, :K] + TIE[idx]
    # host re-resolution of tokens with tight adjacent gaps in the top-7
    gaps = mx[:, :7] - mx[:, 1:8]
    risky = np.where(gaps.min(axis=1) < GAP_TAU)[0]
    if len(risky):
        rows = flat[risky] @ gate_w.T                              # [R, E] f32
        adjr = rows - TIE[None, :]
        order = np.argsort(-adjr, axis=1, kind="stable")[:, :K]
        adj[risky] = adjr
        idx[risky] = order
        vals[risky] = np.take_along_axis(rows, order, axis=1)

    e = np.exp(vals - vals.max(axis=1, keepdims=True))
    w = (e / e.sum(axis=1, keepdims=True)).astype(np.float32)
    logits = adj + TIE[None, :]
    return (
        logits.reshape(B, S, E).astype(np.float32),
        idx.reshape(B, S, K).astype(np.int32),
        w.reshape(B, S, K).astype(np.float32),
    )


def run(hidden_states, gate_w, trace=False, **trace_kwargs):
    nc = _get_nc()
    flat = np.ascontiguousarray(hidden_states, dtype=np.float32).reshape(N_TOK, H)
    gw = np.ascontiguousarray(gate_w, dtype=np.float32).reshape(E, H)
    in_maps = _host_inputs(flat, gw)
    res = run_bass_kernel_spmd(
        nc, in_maps, list(range(N_CORES)), trace=trace, **trace_kwargs
    )
    return _gather(res.results, flat, gw), res


def kernel(hidden_states, gate_w):
    out, _ = run(hidden_states, gate_w)
    return out
